# revision 6
# baseline (speedup 1.0000x reference)
"""Co-Teaching loss kernel for Trainium2 (8 NeuronCores, Bass/Tile).

Strategy
--------
The heavy part is per-sample cross-entropy over two [65536, 1000] f32 logit
tensors (memory-bound: ~0.5 GB of HBM reads).  Observation: the "cross-update"
losses in the reference are just gathers from the already-computed per-sample
loss vectors:

    loss_1_update.mean() = mean_{i in S2} loss_1[i],  S2 = num_keep smallest loss_2
    loss_2_update.mean() = mean_{i in S1} loss_2[i],  S1 = num_keep smallest loss_1

So the device only needs to produce, per row: sum(exp(x)) and x[target]
(loss = ln(sumexp) - x_t; logits ~ N(0,1) so max-subtraction is unnecessary
in f32).  The ln, the top-k selection over a 65536-float vector, and the four
means are cheap host glue.

Device kernel (per core, data-parallel over rows; 8192 rows/core):
  - DMA [128, G*1000] macro-tiles (HWDGE, 2 MB per transfer).
  - ScalarE: exp(x) with accum_out -> per-row sum(exp(x))    (one pass)
  - VectorE: (iota == target) * x with accum_out -> x[target] (one pass)
  Both engines sit below the ~358 GB/s/core HBM roofline; kernel is DMA-bound
  (~177 us/core measured, ~188 us DMA-only in the cost model).
"""

import sys

sys.path.insert(0, "/opt/trn_rl_repo")

import numpy as np

# Problem shape (hardcoded per contract)
N, C = 65536, 1000
NCORES = 8
R = N // NCORES  # 8192 rows per core
P = 128          # SBUF partitions
T = R // P       # 64 row-tiles per net per core
G = 4            # row-groups per DMA macro-tile (2 MB per dma_start)
MT = T // G      # macro-tiles per net

_CACHE = {}


def _build_nc(rows=R, repeat=1, g=G, xin_bufs=6):
    """Build + compile the per-core Bass program. rows must divide into P*g.

    repeat > 1 wraps the whole workload in a runtime loop (same data each
    iteration) — used only by test.py to measure HW exec time through the
    ~80 ms axon dispatch overhead (slope of wall-time vs repeat).
    """
    import concourse.tile as tile
    from concourse import bacc, mybir

    t = rows // P
    mt = t // g

    fp32 = mybir.dt.float32
    Act = mybir.ActivationFunctionType
    Alu = mybir.AluOpType

    nc = bacc.Bacc("TRN2", target_bir_lowering=False, debug=False,
                   num_devices=NCORES)
    y1 = nc.dram_tensor("y1", [rows, C], fp32, kind="ExternalInput").ap()
    y2 = nc.dram_tensor("y2", [rows, C], fp32, kind="ExternalInput").ap()
    tgt = nc.dram_tensor("tgt", [P, t], fp32, kind="ExternalInput").ap()
    iota = nc.dram_tensor("iota", [P, C], fp32, kind="ExternalInput").ap()
    # out[2*net + 0] = per-row sum(exp(x)), out[2*net + 1] = x[target]
    out = nc.dram_tensor("out", [4, P, t], fp32, kind="ExternalOutput").ap()

    with tile.TileContext(nc) as tc:
        with (
            tc.tile_pool(name="xin", bufs=xin_bufs) as xin_pool,
            tc.tile_pool(name="scr", bufs=2) as scr_pool,
            tc.tile_pool(name="consts", bufs=1) as const_pool,
            tc.tile_pool(name="stats", bufs=1) as stats_pool,
        ):
            iota_sb = const_pool.tile([P, C], fp32, tag="iota")
            nc.sync.dma_start(iota_sb[:], iota)
            tgt_sb = const_pool.tile([P, t], fp32, tag="tgt")
            nc.sync.dma_start(tgt_sb[:], tgt)

            def body():
                for net, y in enumerate([y1, y2]):
                    # [mt, P, g, C] view: macro-tile m, partition p = row
                    # m*g*P + gg*P + p, free dims group-major then class.
                    yv = y.rearrange("(m g p) c -> m p g c", g=g, p=P)
                    sums = stats_pool.tile([P, t], fp32, tag=f"sums{net}")
                    xts = stats_pool.tile([P, t], fp32, tag=f"xts{net}")
                    for m in range(mt):
                        x = xin_pool.tile([P, g * C], fp32, tag="xin")
                        xv = x[:].rearrange("p (gg c) -> p gg c", gg=g)
                        nc.sync.dma_start(xv, yv[m])
                        for gg in range(g):
                            j = m * g + gg
                            xs = x[:, gg * C:(gg + 1) * C]
                            es = scr_pool.tile([P, C], fp32, tag="scrA")
                            nc.scalar.activation(es[:], xs, Act.Exp,
                                                 accum_out=sums[:, j:j + 1])
                            ms = scr_pool.tile([P, C], fp32, tag="scrB")
                            nc.vector.scalar_tensor_tensor(
                                ms[:], iota_sb[:], tgt_sb[:, j:j + 1], xs,
                                Alu.is_equal, Alu.mult,
                                accum_out=xts[:, j:j + 1])
                    nc.sync.dma_start(out[2 * net, :, :], sums[:])
                    nc.sync.dma_start(out[2 * net + 1, :, :], xts[:])

            if repeat == 1:
                body()
            else:
                with tc.For_i(0, repeat, 1):
                    body()

    nc.compile()
    return nc


def _get_nc(rows=R, repeat=1, g=G, xin_bufs=6):
    key = (rows, repeat, g, xin_bufs)
    if key not in _CACHE:
        _CACHE[key] = _build_nc(rows, repeat, g, xin_bufs)
    return _CACHE[key]


def make_in_maps(y_1, y_2, targets):
    iota_np = np.ascontiguousarray(
        np.broadcast_to(np.arange(C, dtype=np.float32), (P, C)))
    in_maps = []
    for c in range(NCORES):
        sl = slice(c * R, (c + 1) * R)
        tgt_np = np.ascontiguousarray(
            targets[sl].astype(np.float32).reshape(T, P).T)
        in_maps.append({
            "y1": y_1[sl],
            "y2": y_2[sl],
            "tgt": tgt_np,
            "iota": iota_np,
        })
    return in_maps


def losses_from_outs(outs):
    """outs: list of 8 per-core [4, P, T] arrays -> (loss_1 [N], loss_2 [N])
    as float64."""
    loss_1 = np.empty(N, dtype=np.float64)
    loss_2 = np.empty(N, dtype=np.float64)
    for c in range(NCORES):
        o = outs[c]
        # [p, j] layout = row j*P + p; .T.ravel() -> row-indexed vector
        loss_1[c * R:(c + 1) * R] = (
            np.log(o[0].T.ravel().astype(np.float64))
            - o[1].T.ravel().astype(np.float64))
        loss_2[c * R:(c + 1) * R] = (
            np.log(o[2].T.ravel().astype(np.float64))
            - o[3].T.ravel().astype(np.float64))
    return loss_1, loss_2


def _device_losses(y_1, y_2, targets, trace=False):
    """Run the 8-core SPMD kernel; return (loss_1 [N], loss_2 [N], results)."""
    from concourse.bass_utils import run_bass_kernel_spmd

    nc = _get_nc()
    in_maps = make_in_maps(y_1, y_2, targets)
    res = run_bass_kernel_spmd(nc, in_maps, core_ids=list(range(NCORES)),
                               trace=trace)
    loss_1, loss_2 = losses_from_outs(
        [res.results[c]["out"] for c in range(NCORES)])
    return loss_1, loss_2, res


def kernel(y_1, y_2, targets, num_keep):
    y_1 = np.ascontiguousarray(np.asarray(y_1, dtype=np.float32))
    y_2 = np.ascontiguousarray(np.asarray(y_2, dtype=np.float32))
    targets = np.asarray(targets).astype(np.int64)
    nk = int(num_keep)

    loss_1, loss_2, _ = _device_losses(y_1, y_2, targets)

    ind_1 = np.argpartition(loss_1, nk - 1)[:nk]
    ind_2 = np.argpartition(loss_2, nk - 1)[:nk]
    l1u = loss_1[ind_2].mean()
    l2u = loss_2[ind_1].mean()
    l1m = loss_1.mean()
    l2m = loss_2.mean()
    return np.array([l1u, l2u, l1m, l2m], dtype=np.float32)
